# revision 17
# baseline (speedup 1.0000x reference)
"""CBTree bottom-up fold kernel for 8 trn2 NeuronCores.

Problem: complete 4-ary tree, 9 levels, 87381 nodes in BFS order, d=256.
  leaves (level 8): h = vectors[21845:]
  internal node:    h = tanh(sum_i W_i @ h_child_i + vectors[node])
  where W_i = lc[i]*Wl + rc[i]*Wr,  lc=[1,2/3,1/3,0], rc=[0,1/3,2/3,1].

Strategy (data-parallel over sibling groups):
  - Shard every level contiguously over 8 cores. Children of a core's
    parents are exactly the core's own previous-level outputs, so
    levels 7..2 run with zero communication.
  - One 8KB AllGather of the level-2 states (16 nodes), then every
    core redundantly folds levels 1..0 and writes the root.
  - On chip h lives transposed ([d, nodes], d split into two 128-row
    partition halves) so the tensor engine contracts over d. The
    host-side sharding step hands each core its slices already in this
    [d, nodes] layout (a zero-FLOP relayout done while slicing), so the
    device spends no PE/DVE cycles on transposes: level l is 8
    accumulating 128x128xN matmuls per output half (4 sibling
    positions x 2 d-halves), a 9th identity-weight matmul that adds
    the bias vector in PSUM, and a tanh on the scalar engine writing
    the next level's tile directly.
  - Default dtype is fp16 (fp32 PSUM accumulation): vs fp32r it halves
    every DMA stream and runs full-rate at any moving dim (fp32r drops
    to 1/4 rate below N=256), for ~3.4e-3 scale-relative error vs the
    fp32 reference (fp32r fallback: _mode="f32r", ~1e-3, slower).
  - The root would be an N=1 matmul (invalid ISA), so the last level
    computes 4 replicated copies of the root.
  - The root leaves the device in its transposed on-chip layout
    ([128, 2] fp16); the host transposes it back while unsharding.
"""

import numpy as np

F32 = None  # set on first _lazy_imports()

_BASS = {}


def _lazy_imports():
    global bass, bacc, mybir, tile, make_identity, run_bass_kernel_spmd, F32
    import concourse.bass as bass
    import concourse.mybir as mybir
    from concourse import bacc
    import concourse.tile as tile
    from concourse.masks import make_identity
    from concourse.bass_utils import run_bass_kernel_spmd
    F32 = mybir.dt.float32


N_CORES = 8
D = 256
B = 4
L = 9
SIZES = [B**l for l in range(L)]            # [1,4,16,64,256,1024,4096,16384,65536]
OFFSETS = np.concatenate([[0], np.cumsum(SIZES)])  # [0,1,5,21,85,341,1365,5461,21845,87381]
N_LEAF_CORE = SIZES[8] // N_CORES           # 8192
# local (sharded) levels produce parents at levels 7..2
LOC_LEVELS = [7, 6, 5, 4, 3, 2]
LOC_PAR = {l: SIZES[l] // N_CORES for l in LOC_LEVELS}   # 2048,512,128,32,8,2
N_VECS_LOC = sum(LOC_PAR.values())          # 2730
N_VECS_TAIL = int(OFFSETS[2]) + 4           # 5 real rows + 4x replicated root row


def _build_nc(mode="fp16", WARM=9, CHUNKS=None, CSZ=1024):
    if CHUNKS is None:
        CHUNKS = {7: 256, 6: 128, 5: 64, 4: 16}
    key = ("nc", mode, WARM, tuple(sorted(CHUNKS.items())), CSZ)
    if key in _BASS:
        return _BASS[key]
    nc = bacc.Bacc(num_devices=N_CORES)
    mmdt = {"f32r": mybir.dt.float32r, "fp32": F32,
            "fp16": mybir.dt.float16}[mode]
    dsz = 2 if mode == "fp16" else 4

    # all h/vec tensors arrive transposed: [256 = 2x128 d-rows, n nodes]
    leavesT = nc.declare_dram_parameter("leavesT", [D, N_LEAF_CORE], mmdt, isOutput=False)
    vecs_locT = nc.declare_dram_parameter("vecs_locT", [D, N_VECS_LOC], mmdt, isOutput=False)
    vecs_tailT = nc.declare_dram_parameter("vecs_tailT", [D, N_VECS_TAIL], mmdt, isOutput=False)
    wmat = nc.declare_dram_parameter("wmat", [128, 16 * 128], mmdt, isOutput=False)
    # root in transposed layout: row k, col mh -> component mh*128+k
    out = nc.declare_dram_parameter("out", [128, 2], mmdt, isOutput=True)

    NLOC2 = SIZES[2] // N_CORES                        # 2
    N_L7CH = LOC_PAR[7] // CHUNKS.get(7, 256)          # vloc7 chunk count

    with tile.TileContext(nc) as tc:
        with (
            tc.tile_pool(name="const", bufs=1) as const_pool,
            tc.tile_pool(name="hbuf", bufs=1) as hbuf,
            tc.tile_pool(name="vecp", bufs=8) as vec_pool,
            tc.tile_pool(name="pmm", bufs=6, space="PSUM") as psum_mm,
            tc.tile_pool(name="pwarm", bufs=2, space="PSUM") as psum_warm,
            tc.tile_pool(name="dram", bufs=1, space="DRAM") as dram_pool,
        ):
            ident = const_pool.tile([128, 128], mmdt if mode == "fp16" else F32,
                                    name="ident")
            make_identity(nc, ident)
            # touch Tanh once so Bacc's activation-table load happens during
            # the initial DMA shadow instead of before the first real tanh
            warm_act = const_pool.tile([128, 4], F32, name="warm_act")
            nc.scalar.activation(warm_act[:1, :4], ident[:1, :4],
                                 mybir.ActivationFunctionType.Tanh)

            # leaf states: one tile per kh half (separate tiles keep the
            # per-chunk DMA write regions disjoint for the dep tracker)
            hT8 = [hbuf.tile([128, N_LEAF_CORE], mmdt, name=f"hT8_{kh}",
                             tag=f"hT8_{kh}") for kh in (0, 1)]

            # persistent transposed h states, one tile per (level, d-half)
            def h_tiles(name, n):
                return [hbuf.tile([128, max(n, 1)], mmdt, name=f"{name}_{kh}", tag=f"{name}_{kh}")
                        for kh in (0, 1)]

            hT = {7: h_tiles("hT7", 2048), 6: h_tiles("hT6", 512),
                  5: h_tiles("hT5", 128), 4: h_tiles("hT4", 32),
                  3: h_tiles("hT3", 8)}
            # level-2 state and the gathered level-2 array live packed
            # (both d-halves in one tile) so the AG bounce is 1 DMA each way
            t2p = hbuf.tile([128, 2 * NLOC2], mmdt, name="hT2p", tag="hT2p")
            hT[2] = [t2p[:, 0:NLOC2], t2p[:, NLOC2:2 * NLOC2]]
            h2ap = hbuf.tile([128, 2 * SIZES[2]], mmdt, name="h2allp", tag="h2allp")
            # tail levels (replicated): level-1 and root, packed tiles
            t1p = hbuf.tile([128, 8], mmdt, name="hTt1p", tag="hTt1p")
            hTt1 = [t1p[:, 0:4], t1p[:, 4:8]]
            t0p = hbuf.tile([128, 8], mmdt, name="hTt0p", tag="hTt0p")
            hTt0 = [t0p[:, 0:4], t0p[:, 4:8]]

            wsb = const_pool.tile([128, 16 * 128], mmdt, name="wsb")
            vloc = vec_pool.tile([128, 2, N_VECS_LOC], mmdt, name="vloc",
                                 tag="vloc", bufs=1)
            vtail = vec_pool.tile([128, 2, N_VECS_TAIL], mmdt, name="vtail",
                                  tag="vtail", bufs=1)

            csz = CSZ
            ntot = N_LEAF_CORE // csz

            def leaf_chunk(c):
                for kh in (0, 1):
                    nc.sync.dma_start(
                        hT8[kh][:, c * csz:(c + 1) * csz],
                        leavesT[kh * 128:(kh + 1) * 128, c * csz:(c + 1) * csz])

            vloc_src = vecs_locT[:].rearrange("(mh k) n -> k mh n", mh=2)
            CH7 = CHUNKS.get(7, 256) if mode == "fp16" else 512
            V7 = LOC_PAR[7] // N_L7CH                  # vloc7 chunk width

            def vloc7_chunk(c):
                nc.scalar.dma_start(vloc[:, :, c * V7:(c + 1) * V7],
                                    vloc_src[:, :, c * V7:(c + 1) * V7])

            def load_vloc6():
                nc.scalar.dma_start(vloc[:, :, 2048:2560],
                                    vloc_src[:, :, 2048:2560])

            def load_vloc_low():
                nc.scalar.dma_start(vloc[:, :, 2560:],
                                    vloc_src[:, :, 2560:])

            def load_vtail():
                nc.scalar.dma_start(
                    vtail[:],
                    vecs_tailT[:].rearrange("(mh k) n -> k mh n", mh=2))

            def pe_warm(n):
                # burn the PE p-state ramp on the identity tile while the
                # first leaf/weight DMAs stream in
                rhs = ident[:, :].unsqueeze(1).broadcast_to([128, 4, 128])
                for _ in range(n):
                    scr = psum_warm.tile([128, 512], F32, name="ps_w", tag="w")
                    nc.tensor.matmul(scr[:, :512], ident[:], rhs,
                                     start=True, stop=True)

            # ---- shared level routine ----
            def do_level(child, n_par, vec_tile, vec_col0, hT_out,
                         chunk_prologue=None, chunk=512, rview_fn=None):
                if rview_fn is None:
                    rview = [child[kh][:, :4 * n_par].rearrange(
                        "k (p four) -> k p four", four=4) for kh in (0, 1)]

                    def rview_fn(kh, c0, N, i):
                        return rview[kh][:, c0:c0 + N, i]
                for c0 in range(0, n_par, chunk):
                    if chunk_prologue is not None:
                        chunk_prologue(c0)
                    N = min(chunk, n_par - c0)
                    vts = [vec_tile[:, mh, vec_col0 + c0: vec_col0 + c0 + N]
                           for mh in (0, 1)]
                    for mh in (0, 1):
                        ps = psum_mm.tile([128, 512], F32, name="ps_mm", tag="mm")
                        for i in range(4):
                            for kh in (0, 1):
                                blk = mh * 8 + i * 2 + kh
                                w = wsb[:, blk * 128:(blk + 1) * 128]
                                nc.tensor.matmul(ps[:, :N], w,
                                                 rview_fn(kh, c0, N, i),
                                                 start=(i == 0 and kh == 0),
                                                 stop=False)
                        nc.tensor.matmul(ps[:, :N], ident[:],
                                         vts[mh][:, :N], start=False, stop=True)
                        nc.scalar.activation(hT_out[mh][:, c0:c0 + N], ps[:, :N],
                                             mybir.ActivationFunctionType.Tanh)

            # ---- software-pipelined emission over levels 7..2: a chunk of
            # level l is emitted as soon as its children (level l+1 outputs)
            # are a chunk ahead, so after the last leaf DMA only the small
            # final-chunk cascade remains on the critical path. All HBM
            # traffic is issued in exact consumption order with a two-chunk
            # lookahead so the serialized DMA stream delivers each chunk's
            # bytes (children for level 7, bias vecs for all) just in time.
            vcol0 = {}
            acc = 0
            for l in LOC_LEVELS:
                vcol0[l] = acc
                acc += LOC_PAR[l]
            child_of = {l: (hT8 if l == 7 else hT[l + 1]) for l in LOC_LEVELS}
            rviews = {l: [child_of[l][kh][:, :4 * LOC_PAR[l]].rearrange(
                "k (p four) -> k p four", four=4) for kh in (0, 1)]
                for l in LOC_LEVELS}
            CH = {l: (CHUNKS.get(l, 512) if mode == "fp16" else 512)
                  for l in LOC_LEVELS}
            assert csz == 4 * CH[7], "one leaf chunk per level-7 chunk"

            # plan the emission order (pure planning, no instruction emission)
            pending = {l: 0 for l in LOC_LEVELS}

            def ready(l, need_par_cols):
                if l == 7:
                    return True
                cl = l + 1
                if pending[cl] >= LOC_PAR[cl]:
                    return True
                # children needed plus one chunk of slack at the child level
                return pending[cl] >= need_par_cols + CH[cl]

            order = []
            while any(pending[l] < LOC_PAR[l] for l in LOC_LEVELS):
                for l in sorted(LOC_LEVELS):          # shallowest first
                    if pending[l] >= LOC_PAR[l]:
                        continue
                    N = min(CH[l], LOC_PAR[l] - pending[l])
                    if ready(l, 4 * (pending[l] + N)):
                        order.append((l, pending[l], N))
                        pending[l] += N
                        break
                else:
                    raise AssertionError("pipeline stuck")

            def issue_group(k):
                # HBM bytes consumed by planned chunk k, in consumption order
                if k == len(order):
                    load_vtail()
                    return
                if k > len(order):
                    return
                l, c0, N = order[k]
                if l == 7:
                    leaf_chunk(c0 // CH[7])
                nc.scalar.dma_start(
                    vloc[:, :, vcol0[l] + c0: vcol0[l] + c0 + N],
                    vloc_src[:, :, vcol0[l] + c0: vcol0[l] + c0 + N])

            def emit_chunk(l, c0, N):
                vts = [vloc[:, mh, vcol0[l] + c0: vcol0[l] + c0 + N]
                       for mh in (0, 1)]
                for mh in (0, 1):
                    ps = psum_mm.tile([128, 512], F32, name="ps_mm", tag="mm")
                    for i in range(4):
                        for kh in (0, 1):
                            blk = mh * 8 + i * 2 + kh
                            nc.tensor.matmul(
                                ps[:, :N], wsb[:, blk * 128:(blk + 1) * 128],
                                rviews[l][kh][:, c0:c0 + N, i],
                                start=(i == 0 and kh == 0), stop=False)
                    nc.tensor.matmul(ps[:, :N], ident[:], vts[mh][:, :N],
                                     start=False, stop=True)
                    nc.scalar.activation(hT[l][mh][:, c0:c0 + N], ps[:, :N],
                                         mybir.ActivationFunctionType.Tanh)

            # startup: first chunk's bytes, weights, second chunk's bytes;
            # PE warms on ident meanwhile
            issue_group(0)
            nc.scalar.dma_start(wsb[:, :8 * 128], wmat[:, :8 * 128])
            issue_group(1)
            nc.scalar.dma_start(wsb[:, 8 * 128:], wmat[:, 8 * 128:])
            pe_warm(WARM)

            for k, (l, c0, N) in enumerate(order):
                issue_group(k + 2)
                emit_chunk(l, c0, N)

            # ---- AllGather of level-2 states, transposed layout ----
            # per-rank bounce [256 d, 2 nodes]; gathered [8*256, 2]
            cc_in = dram_pool.tile([D, NLOC2], mmdt, name="cc_in")
            cc_out = dram_pool.tile([N_CORES * D, NLOC2], mmdt,
                                    name="cc_out")
            nc.sync.dma_start(
                cc_in[:].rearrange("(kh k) n -> k kh n", kh=2),
                t2p[:].rearrange("k (kh n) -> k kh n", kh=2))
            nc.gpsimd.collective_compute(
                "AllGather", mybir.AluOpType.bypass,
                replica_groups=[list(range(N_CORES))],
                ins=[cc_in.opt()], outs=[cc_out.opt()])
            # single return DMA: keep the gathered layout (r kh n)-major in
            # columns — h2ap col = r*4 + kh*2 + n, fully contiguous dst
            nc.sync.dma_start(
                h2ap[:].rearrange("k (rk n) -> k rk n", rk=2 * N_CORES),
                cc_out[:].rearrange("(rk k) n -> k rk n", rk=2 * N_CORES))

            # ---- replicated tail: level 1, then 4 copies of the root ----
            # level-2 node j=(r*2+n) at sibling pos i of parent p: j=4p+i,
            # h2ap col = 8p + 4*(i//2) + 2*kh + i%2
            h2v = h2ap[:].rearrange("k (p i2 khd i1) -> k p i2 khd i1",
                                    p=4, i2=2, khd=2)

            def rv_l1(kh, c0, N, i):
                return h2v[:, c0:c0 + N, i // 2, kh, i % 2]

            do_level(None, SIZES[1], vtail, 1, hTt1, rview_fn=rv_l1)

            def rv_root(kh, c0, N, i):
                return hTt1[kh][:, i:i + 1].broadcast_to([128, N])

            do_level(None, 4, vtail, int(OFFSETS[2]), hTt0,
                     rview_fn=rv_root)

            # root col 0 of each mh half, transposed layout; host undoes it
            nc.sync.dma_start(
                out[:],
                t0p[:].rearrange("k (mh c) -> k mh c", mh=2)[:, :, 0])

    nc.finalize()
    _BASS[key] = nc
    return nc


def _prep_inputs(vectors, Wl, Wr, mode="fp16"):
    vectors = np.asarray(vectors, dtype=np.float32)
    Wl = np.asarray(Wl, dtype=np.float32)
    Wr = np.asarray(Wr, dtype=np.float32)

    ind = np.arange(1, B + 1, dtype=np.float32)
    lc = (B - ind) / (B - 1)
    rc = (ind - 1) / (B - 1)
    # W_t[i] = W_i.T laid out [k', (mh, i, kh, m')] for SBUF [128, 2048]
    Wt = np.stack([lc[i] * Wl.T + rc[i] * Wr.T for i in range(B)])  # [4, 256k, 256m]
    W5 = Wt.reshape(4, 2, 128, 2, 128)            # [i, kh, k', mh, m']
    halves = [W5[:, :, :, mh, :].reshape(4, 2, 128, 128)
              .transpose(2, 0, 1, 3).reshape(128, 8 * 128) for mh in (0, 1)]
    wmat = np.ascontiguousarray(np.concatenate(halves, axis=1),
                                dtype=np.float32)

    # one transposed copy of the node array; all per-core slices are views
    # into it laid out [d, nodes] (part of sharding, no arithmetic)
    vecsT = np.ascontiguousarray(vectors.T)                      # [256, 87381]
    vecs_tailT = np.ascontiguousarray(
        np.concatenate([vecsT[:, :int(OFFSETS[2])],
                        np.repeat(vecsT[:, 0:1], 4, axis=1)], axis=1))
    import ml_dtypes  # noqa: F401  (fp16 path uses numpy's float16)
    hdt = np.float16 if mode == "fp16" else np.float32
    in_maps = []
    for c in range(N_CORES):
        o8 = int(OFFSETS[8])
        leavesT_c = vecsT[:, o8 + c * N_LEAF_CORE: o8 + (c + 1) * N_LEAF_CORE]
        loc_parts = []
        for l in LOC_LEVELS:
            npl = LOC_PAR[l]
            o = int(OFFSETS[l])
            loc_parts.append(vecsT[:, o + c * npl: o + (c + 1) * npl])
        im = {
            "leavesT": np.ascontiguousarray(leavesT_c).astype(hdt),
            "vecs_locT": np.ascontiguousarray(
                np.concatenate(loc_parts, axis=1)).astype(hdt),
            "vecs_tailT": vecs_tailT.astype(hdt),
            "wmat": wmat.astype(hdt),
        }
        in_maps.append(im)
    return in_maps


def kernel(vectors, Wl, Wr, branching, n_levels, _mode="fp16"):
    _lazy_imports()
    assert int(branching) == B and int(n_levels) == L
    vectors = np.asarray(vectors)
    assert vectors.shape == (int(OFFSETS[L]), D), vectors.shape

    nc = _build_nc(mode=_mode)
    in_maps = _prep_inputs(vectors, Wl, Wr, mode=_mode)
    try:
        res = run_bass_kernel_spmd(nc, in_maps, core_ids=list(range(N_CORES)),
                                   trace=False)
    except Exception:
        # transient device hiccups (e.g. NRT_EXEC_UNIT_UNRECOVERABLE right
        # after another process released the cores) clear on a retry
        res = run_bass_kernel_spmd(nc, in_maps, core_ids=list(range(N_CORES)),
                                   trace=False)
    root = res.results[0]["out"]
    # undo the on-chip transposed layout: [128 k, 2 mh] -> d = mh*128 + k
    return np.asarray(root).astype(np.float32).T.reshape(1, D)


# revision 19
# speedup vs baseline: 1.0849x; 1.0849x over previous
"""CBTree bottom-up fold kernel for 8 trn2 NeuronCores.

Problem: complete 4-ary tree, 9 levels, 87381 nodes in BFS order, d=256.
  leaves (level 8): h = vectors[21845:]
  internal node:    h = tanh(sum_i W_i @ h_child_i + vectors[node])
  where W_i = lc[i]*Wl + rc[i]*Wr,  lc=[1,2/3,1/3,0], rc=[0,1/3,2/3,1].

Strategy (data-parallel over sibling groups):
  - Shard every level contiguously over 8 cores. Children of a core's
    parents are exactly the core's own previous-level outputs, so
    levels 7..2 run with zero communication.
  - One 8KB AllGather of the level-2 states (16 nodes), then every
    core redundantly folds levels 1..0 and writes the root.
  - On chip h lives transposed ([d, nodes], d split into two 128-row
    partition halves) so the tensor engine contracts over d. The
    host-side sharding step hands each core its slices already in this
    [d, nodes] layout (a zero-FLOP relayout done while slicing), so the
    device spends no PE/DVE cycles on transposes: level l is 8
    accumulating 128x128xN matmuls per output half (4 sibling
    positions x 2 d-halves), a 9th identity-weight matmul that adds
    the bias vector in PSUM, and a tanh on the scalar engine writing
    the next level's tile directly.
  - Default dtype is fp16 (fp32 PSUM accumulation): vs fp32r it halves
    every DMA stream and runs full-rate at any moving dim (fp32r drops
    to 1/4 rate below N=256), for ~3.4e-3 scale-relative error vs the
    fp32 reference (fp32r fallback: _mode="f32r", ~1e-3, slower).
  - The root would be an N=1 matmul (invalid ISA), so the last level
    computes 4 replicated copies of the root.
  - The root leaves the device in its transposed on-chip layout
    ([128, 2] fp16); the host transposes it back while unsharding.
"""

import numpy as np

F32 = None  # set on first _lazy_imports()

_BASS = {}


def _lazy_imports():
    global bass, bacc, mybir, tile, make_identity, run_bass_kernel_spmd, F32
    import concourse.bass as bass
    import concourse.mybir as mybir
    from concourse import bacc
    import concourse.tile as tile
    from concourse.masks import make_identity
    from concourse.bass_utils import run_bass_kernel_spmd
    F32 = mybir.dt.float32


N_CORES = 8
D = 256
B = 4
L = 9
SIZES = [B**l for l in range(L)]            # [1,4,16,64,256,1024,4096,16384,65536]
OFFSETS = np.concatenate([[0], np.cumsum(SIZES)])  # [0,1,5,21,85,341,1365,5461,21845,87381]
N_LEAF_CORE = SIZES[8] // N_CORES           # 8192
# local (sharded) levels produce parents at levels 7..2
LOC_LEVELS = [7, 6, 5, 4, 3, 2]
LOC_PAR = {l: SIZES[l] // N_CORES for l in LOC_LEVELS}   # 2048,512,128,32,8,2
N_VECS_LOC = sum(LOC_PAR.values())          # 2730
N_VECS_TAIL = int(OFFSETS[2]) + 4           # 5 real rows + 4x replicated root row


def _build_nc(mode="fp16", WARM=9, CHUNKS=None, CSZ=1024):
    if CHUNKS is None:
        CHUNKS = {7: 256, 6: 128, 5: 64, 4: 16}
    key = ("nc", mode, WARM, tuple(sorted(CHUNKS.items())), CSZ)
    if key in _BASS:
        return _BASS[key]
    nc = bacc.Bacc(num_devices=N_CORES)
    mmdt = {"f32r": mybir.dt.float32r, "fp32": F32,
            "fp16": mybir.dt.float16}[mode]
    dsz = 2 if mode == "fp16" else 4

    # all h/vec tensors arrive transposed: [256 = 2x128 d-rows, n nodes]
    leavesT = nc.declare_dram_parameter("leavesT", [D, N_LEAF_CORE], mmdt, isOutput=False)
    vecs_locT = nc.declare_dram_parameter("vecs_locT", [D, N_VECS_LOC], mmdt, isOutput=False)
    vecs_tailT = nc.declare_dram_parameter("vecs_tailT", [D, N_VECS_TAIL], mmdt, isOutput=False)
    wmat = nc.declare_dram_parameter("wmat", [128, 16 * 128], mmdt, isOutput=False)
    # root in transposed layout: row k, col mh -> component mh*128+k
    out = nc.declare_dram_parameter("out", [128, 2], mmdt, isOutput=True)

    NLOC2 = SIZES[2] // N_CORES                        # 2
    N_L7CH = LOC_PAR[7] // CHUNKS.get(7, 256)          # vloc7 chunk count

    with tile.TileContext(nc) as tc:
        with (
            tc.tile_pool(name="const", bufs=1) as const_pool,
            tc.tile_pool(name="hbuf", bufs=1) as hbuf,
            tc.tile_pool(name="vecp", bufs=8) as vec_pool,
            tc.tile_pool(name="pmm", bufs=6, space="PSUM") as psum_mm,
            tc.tile_pool(name="pwarm", bufs=2, space="PSUM") as psum_warm,
            tc.tile_pool(name="dram", bufs=1, space="DRAM") as dram_pool,
        ):
            ident = const_pool.tile([128, 128], mmdt if mode == "fp16" else F32,
                                    name="ident")
            make_identity(nc, ident)
            # touch Tanh once so Bacc's activation-table load happens during
            # the initial DMA shadow instead of before the first real tanh
            warm_act = const_pool.tile([128, 4], F32, name="warm_act")
            nc.scalar.activation(warm_act[:1, :4], ident[:1, :4],
                                 mybir.ActivationFunctionType.Tanh)

            # leaf states: one tile per kh half (separate tiles keep the
            # per-chunk DMA write regions disjoint for the dep tracker)
            hT8 = [hbuf.tile([128, N_LEAF_CORE], mmdt, name=f"hT8_{kh}",
                             tag=f"hT8_{kh}") for kh in (0, 1)]

            # persistent transposed h states, one tile per (level, d-half)
            def h_tiles(name, n):
                return [hbuf.tile([128, max(n, 1)], mmdt, name=f"{name}_{kh}", tag=f"{name}_{kh}")
                        for kh in (0, 1)]

            hT = {7: h_tiles("hT7", 2048), 6: h_tiles("hT6", 512),
                  5: h_tiles("hT5", 128), 4: h_tiles("hT4", 32),
                  3: h_tiles("hT3", 8)}
            # level-2 state and the gathered level-2 array live packed
            # (both d-halves in one tile) so the AG bounce is 1 DMA each way
            t2p = hbuf.tile([128, 2 * NLOC2], mmdt, name="hT2p", tag="hT2p")
            hT[2] = [t2p[:, 0:NLOC2], t2p[:, NLOC2:2 * NLOC2]]
            h2ap = hbuf.tile([128, 2 * SIZES[2]], mmdt, name="h2allp", tag="h2allp")
            # tail levels (replicated): level-1 and root, packed tiles
            t1p = hbuf.tile([128, 8], mmdt, name="hTt1p", tag="hTt1p")
            hTt1 = [t1p[:, 0:4], t1p[:, 4:8]]
            t0p = hbuf.tile([128, 8], mmdt, name="hTt0p", tag="hTt0p")
            hTt0 = [t0p[:, 0:4], t0p[:, 4:8]]

            wsb = const_pool.tile([128, 16 * 128], mmdt, name="wsb")
            vloc = vec_pool.tile([128, 2, N_VECS_LOC], mmdt, name="vloc",
                                 tag="vloc", bufs=1)
            vtail = vec_pool.tile([128, 2, N_VECS_TAIL], mmdt, name="vtail",
                                  tag="vtail", bufs=1)

            csz = CSZ
            ntot = N_LEAF_CORE // csz

            def leaf_chunk(c):
                for kh in (0, 1):
                    nc.sync.dma_start(
                        hT8[kh][:, c * csz:(c + 1) * csz],
                        leavesT[kh * 128:(kh + 1) * 128, c * csz:(c + 1) * csz])

            vloc_src = vecs_locT[:].rearrange("(mh k) n -> k mh n", mh=2)
            CH7 = CHUNKS.get(7, 256) if mode == "fp16" else 512
            V7 = LOC_PAR[7] // N_L7CH                  # vloc7 chunk width

            def vloc7_chunk(c):
                nc.scalar.dma_start(vloc[:, :, c * V7:(c + 1) * V7],
                                    vloc_src[:, :, c * V7:(c + 1) * V7])

            def load_vloc_rest():
                nc.scalar.dma_start(vloc[:, :, LOC_PAR[7]:],
                                    vloc_src[:, :, LOC_PAR[7]:])

            def load_vtail():
                nc.scalar.dma_start(
                    vtail[:],
                    vecs_tailT[:].rearrange("(mh k) n -> k mh n", mh=2))

            def pe_warm(n):
                # burn the PE p-state ramp on the identity tile while the
                # first leaf/weight DMAs stream in
                rhs = ident[:, :].unsqueeze(1).broadcast_to([128, 4, 128])
                for _ in range(n):
                    scr = psum_warm.tile([128, 512], F32, name="ps_w", tag="w")
                    nc.tensor.matmul(scr[:, :512], ident[:], rhs,
                                     start=True, stop=True)

            # ---- shared level routine ----
            def do_level(child, n_par, vec_tile, vec_col0, hT_out,
                         chunk_prologue=None, chunk=512, rview_fn=None):
                if rview_fn is None:
                    rview = [child[kh][:, :4 * n_par].rearrange(
                        "k (p four) -> k p four", four=4) for kh in (0, 1)]

                    def rview_fn(kh, c0, N, i):
                        return rview[kh][:, c0:c0 + N, i]
                for c0 in range(0, n_par, chunk):
                    if chunk_prologue is not None:
                        chunk_prologue(c0)
                    N = min(chunk, n_par - c0)
                    vts = [vec_tile[:, mh, vec_col0 + c0: vec_col0 + c0 + N]
                           for mh in (0, 1)]
                    for mh in (0, 1):
                        ps = psum_mm.tile([128, 512], F32, name="ps_mm", tag="mm")
                        for i in range(4):
                            for kh in (0, 1):
                                blk = mh * 8 + i * 2 + kh
                                w = wsb[:, blk * 128:(blk + 1) * 128]
                                nc.tensor.matmul(ps[:, :N], w,
                                                 rview_fn(kh, c0, N, i),
                                                 start=(i == 0 and kh == 0),
                                                 stop=False)
                        nc.tensor.matmul(ps[:, :N], ident[:],
                                         vts[mh][:, :N], start=False, stop=True)
                        nc.scalar.activation(hT_out[mh][:, c0:c0 + N], ps[:, :N],
                                             mybir.ActivationFunctionType.Tanh)

            # ---- startup DMA order: leaf chunk 0, weights, first vloc7
            # chunks, leaf chunk 1; PE warms on ident meanwhile
            leaf_chunk(0)
            nc.scalar.dma_start(wsb[:, :8 * 128], wmat[:, :8 * 128])
            vloc7_chunk(0)
            nc.scalar.dma_start(wsb[:, 8 * 128:], wmat[:, 8 * 128:])
            if ntot > 1:
                leaf_chunk(1)
            if N_L7CH > 1:
                vloc7_chunk(1)
            pe_warm(WARM)

            # ---- local levels 7..2 ----
            # leaf-DMA chunks consumed per level-7 compute chunk
            R = max(1, 4 * CH7 // csz)

            def prologue7(c0):
                ci = c0 // CH7
                for c in range(R * (ci + 2), min(R * (ci + 3), ntot)):
                    leaf_chunk(c)
                if ci + 2 < N_L7CH:
                    vloc7_chunk(ci + 2)
                if ci == 2:
                    load_vloc_rest()

            col0 = 0
            child = hT8
            for l in LOC_LEVELS:
                do_level(child, LOC_PAR[l], vloc, col0, hT[l],
                         chunk_prologue=prologue7 if l == 7 else None,
                         chunk=CHUNKS.get(l, 512) if mode == "fp16" else 512)
                col0 += LOC_PAR[l]
                child = hT[l]

            load_vtail()

            # ---- AllGather of level-2 states, transposed layout ----
            # per-rank bounce [256 d, 2 nodes]; gathered [8*256, 2]
            cc_in = dram_pool.tile([D, NLOC2], mmdt, name="cc_in")
            cc_out = dram_pool.tile([N_CORES * D, NLOC2], mmdt,
                                    name="cc_out")
            nc.sync.dma_start(
                cc_in[:].rearrange("(kh k) n -> k kh n", kh=2),
                t2p[:].rearrange("k (kh n) -> k kh n", kh=2))
            nc.gpsimd.collective_compute(
                "AllGather", mybir.AluOpType.bypass,
                replica_groups=[list(range(N_CORES))],
                ins=[cc_in.opt()], outs=[cc_out.opt()])
            # single return DMA: keep the gathered layout (r kh n)-major in
            # columns — h2ap col = r*4 + kh*2 + n, fully contiguous dst
            nc.sync.dma_start(
                h2ap[:].rearrange("k (rk n) -> k rk n", rk=2 * N_CORES),
                cc_out[:].rearrange("(rk k) n -> k rk n", rk=2 * N_CORES))

            # ---- replicated tail: level 1, then 4 copies of the root ----
            # level-2 node j=(r*2+n) at sibling pos i of parent p: j=4p+i,
            # h2ap col = 8p + 4*(i//2) + 2*kh + i%2
            h2v = h2ap[:].rearrange("k (p i2 khd i1) -> k p i2 khd i1",
                                    p=4, i2=2, khd=2)

            def rv_l1(kh, c0, N, i):
                return h2v[:, c0:c0 + N, i // 2, kh, i % 2]

            do_level(None, SIZES[1], vtail, 1, hTt1, rview_fn=rv_l1)

            def rv_root(kh, c0, N, i):
                return hTt1[kh][:, i:i + 1].broadcast_to([128, N])

            do_level(None, 4, vtail, int(OFFSETS[2]), hTt0,
                     rview_fn=rv_root)

            # root col 0 of each mh half, transposed layout; host undoes it
            nc.sync.dma_start(
                out[:],
                t0p[:].rearrange("k (mh c) -> k mh c", mh=2)[:, :, 0])

    nc.finalize()
    _BASS[key] = nc
    return nc


def _prep_inputs(vectors, Wl, Wr, mode="fp16"):
    vectors = np.asarray(vectors, dtype=np.float32)
    Wl = np.asarray(Wl, dtype=np.float32)
    Wr = np.asarray(Wr, dtype=np.float32)

    ind = np.arange(1, B + 1, dtype=np.float32)
    lc = (B - ind) / (B - 1)
    rc = (ind - 1) / (B - 1)
    # W_t[i] = W_i.T laid out [k', (mh, i, kh, m')] for SBUF [128, 2048]
    Wt = np.stack([lc[i] * Wl.T + rc[i] * Wr.T for i in range(B)])  # [4, 256k, 256m]
    W5 = Wt.reshape(4, 2, 128, 2, 128)            # [i, kh, k', mh, m']
    halves = [W5[:, :, :, mh, :].reshape(4, 2, 128, 128)
              .transpose(2, 0, 1, 3).reshape(128, 8 * 128) for mh in (0, 1)]
    wmat = np.ascontiguousarray(np.concatenate(halves, axis=1),
                                dtype=np.float32)

    # one transposed copy of the node array; all per-core slices are views
    # into it laid out [d, nodes] (part of sharding, no arithmetic)
    vecsT = np.ascontiguousarray(vectors.T)                      # [256, 87381]
    vecs_tailT = np.ascontiguousarray(
        np.concatenate([vecsT[:, :int(OFFSETS[2])],
                        np.repeat(vecsT[:, 0:1], 4, axis=1)], axis=1))
    import ml_dtypes  # noqa: F401  (fp16 path uses numpy's float16)
    hdt = np.float16 if mode == "fp16" else np.float32
    in_maps = []
    for c in range(N_CORES):
        o8 = int(OFFSETS[8])
        leavesT_c = vecsT[:, o8 + c * N_LEAF_CORE: o8 + (c + 1) * N_LEAF_CORE]
        loc_parts = []
        for l in LOC_LEVELS:
            npl = LOC_PAR[l]
            o = int(OFFSETS[l])
            loc_parts.append(vecsT[:, o + c * npl: o + (c + 1) * npl])
        im = {
            "leavesT": np.ascontiguousarray(leavesT_c).astype(hdt),
            "vecs_locT": np.ascontiguousarray(
                np.concatenate(loc_parts, axis=1)).astype(hdt),
            "vecs_tailT": vecs_tailT.astype(hdt),
            "wmat": wmat.astype(hdt),
        }
        in_maps.append(im)
    return in_maps


def kernel(vectors, Wl, Wr, branching, n_levels, _mode="fp16"):
    _lazy_imports()
    assert int(branching) == B and int(n_levels) == L
    vectors = np.asarray(vectors)
    assert vectors.shape == (int(OFFSETS[L]), D), vectors.shape

    nc = _build_nc(mode=_mode)
    in_maps = _prep_inputs(vectors, Wl, Wr, mode=_mode)
    try:
        res = run_bass_kernel_spmd(nc, in_maps, core_ids=list(range(N_CORES)),
                                   trace=False)
    except Exception:
        # transient device hiccups (e.g. NRT_EXEC_UNIT_UNRECOVERABLE right
        # after another process released the cores) clear on a retry
        res = run_bass_kernel_spmd(nc, in_maps, core_ids=list(range(N_CORES)),
                                   trace=False)
    root = res.results[0]["out"]
    # undo the on-chip transposed layout: [128 k, 2 mh] -> d = mh*128 + k
    return np.asarray(root).astype(np.float32).T.reshape(1, D)
